# revision 2
# baseline (speedup 1.0000x reference)
"""Trainium2 Bass kernel for ConvolutionalAttention.

Per-batch math (B=8, S=2048, D=1024):
    qkv = x @ W_qkv + b_qkv ; q,k,v = split(qkv)
    A   = causal_softmax(q @ k.T / sqrt(D))
    C   = w0*v_prev + w1*v + w2*v_next        (depthwise 3-tap conv over seq)
    out = A @ C - diag(A) * (w2 * v_next)

Sharding: data-parallel over batch, one batch element per NeuronCore (8 cores).

Per-core pipeline (all in transposed [feature, seq] layout except the A@C
matmul, which needs j on partitions — scoresT already has it):
  P0: xT = transpose(x) via PE transposes                  [D,S] fp32r
  P1: qkvT = W.T @ xT + b (PE, fp32r), spill qT/kT/vT to DRAM
  P2: conv in vT layout (per-partition tap weights), D2T = w0*v_prev+w1*v,
      CT = D2T + w2*v_next; PE-transpose both to natural [S,D], spill.
  P3: per 256-wide i-chunk: scoresT tiles (PE) -> exp (ACT, no max needed:
      |scores|<~7) -> AV matmuls vs C, diagonal tile split into strict-mask
      part vs C plus diagonal-only part vs D2 (gives the -diagA*w2*v_next
      correction exactly); softmax denom via E.T @ ones matmuls; final
      1/denom as per-partition scale on the PSUM->SBUF copy.
"""
import sys

if "/opt/trn_rl_repo" not in sys.path:
    sys.path.insert(0, "/opt/trn_rl_repo")

from contextlib import ExitStack

import numpy as np

import concourse.bass as bass
import concourse.mybir as mybir
import concourse.tile as tile
from concourse import bacc
from concourse.bass_utils import run_bass_kernel_spmd
from concourse.masks import make_identity, make_upper_triangular

F32 = mybir.dt.float32
F32R = mybir.dt.float32r
AF = mybir.ActivationFunctionType
OP = mybir.AluOpType

P = 128
S = 2048
D = 1024
DT = D // P          # 8 d-tiles
ST = S // P          # 16 s-tiles
CB = 3 * D // P      # 24 qkv channel blocks
IC = 256             # scores i-chunk width
NI = S // IC         # 8 chunks
SCALE = 1.0 / 32.0   # 1/sqrt(D)


def build():
    nc = bacc.Bacc("TRN2", target_bir_lowering=False, debug=False)
    x_d = nc.dram_tensor("x", [S, D], F32, kind="ExternalInput").ap()
    W_d = nc.dram_tensor("W", [D, 3 * D], F32, kind="ExternalInput").ap()
    b_d = nc.dram_tensor("b", [3 * D], F32, kind="ExternalInput").ap()
    wc_d = nc.dram_tensor("wc", [D, 3], F32, kind="ExternalInput").ap()
    out_d = nc.dram_tensor("out", [S, D], F32, kind="ExternalOutput").ap()

    with tile.TileContext(nc) as tc, ExitStack() as ctx:
        dram = ctx.enter_context(tc.tile_pool(name="dram", bufs=1, space="DRAM"))
        qT_dr = dram.tile([D, S], F32R)
        kT_dr = dram.tile([D, S], F32R)
        vT_dr = dram.tile([D, S], F32)
        C_dr = dram.tile([S, D], F32R)
        D2_dr = dram.tile([S, D], F32R)

        const = ctx.enter_context(tc.tile_pool(name="const", bufs=1))
        ident = const.tile([P, P], F32)
        make_identity(nc, ident[:])
        strictm = const.tile([P, P], F32)  # 1 where j < i (partition=j, free=i)
        make_upper_triangular(nc, strictm[:], val=1.0, diag=False)
        identm = const.tile([P, P], F32)   # 1 on diagonal
        make_identity(nc, identm[:])
        ones_f = const.tile([P, 2], F32)
        nc.gpsimd.memset(ones_f[:], 1.0)
        ones_r = const.tile([P, 2], F32R)
        nc.vector.tensor_copy(ones_r[:], ones_f[:])
        bcols = const.tile([P, CB], F32)
        nc.sync.dma_start(bcols[:], b_d.rearrange("(cb p) -> p cb", p=P))
        wcols = const.tile([P, DT * 3], F32)
        nc.sync.dma_start(wcols[:].rearrange("p (dt t) -> p dt t", t=3),
                          wc_d.rearrange("(dt p) t -> p dt t", p=P))

        # ---------------- Phase 0+1: xT, then QKV projection ----------------
        with tc.tile_pool(name="xtr", bufs=1) as xtrp:
            xTr = xtrp.tile([P, DT * S], F32R)  # [d, s] transposed x, rounded
            with tc.tile_pool(name="ph0", bufs=3) as ph0, \
                 tc.tile_pool(name="ph0ps", bufs=4, space="PSUM") as ph0ps:
                for st in range(ST):
                    xin = ph0.tile([P, D], F32, tag="xin")
                    nc.sync.dma_start(xin[:], x_d[st * P:(st + 1) * P, :])
                    for dh in range(2):
                        ps = ph0ps.tile([P, 512], F32, tag="tps")
                        for q in range(4):
                            dt_ = dh * 4 + q
                            nc.tensor.transpose(
                                ps[:, q * P:(q + 1) * P],
                                xin[:, dt_ * P:(dt_ + 1) * P], ident[:])
                        dst3 = xTr[:].rearrange("p (d s) -> p d s", d=DT)[
                            :, dh * 4:(dh + 1) * 4, st * P:(st + 1) * P]
                        src3 = ps[:].rearrange("p (q s) -> p q s", q=4)
                        nc.vector.tensor_copy(dst3, src3)

            with tc.tile_pool(name="wr", bufs=1) as wrp, \
                 tc.tile_pool(name="ws", bufs=2) as ws, \
                 tc.tile_pool(name="qksb", bufs=3) as qksb, \
                 tc.tile_pool(name="qkps", bufs=2, space="PSUM") as qkps:
                for g in range(2):
                    Wr = wrp.tile([P, DT * 1536], F32R, tag="wr")
                    for dt_ in range(DT):
                        wf = ws.tile([P, 1536], F32, tag="wf")
                        nc.sync.dma_start(
                            wf[:], W_d[dt_ * P:(dt_ + 1) * P,
                                       g * 1536:(g + 1) * 1536])
                        nc.vector.tensor_copy(
                            Wr[:, dt_ * 1536:(dt_ + 1) * 1536], wf[:])
                    for cbl in range(12):
                        cb = g * 12 + cbl
                        ps = qkps.tile([P, S], F32, tag="qkps")  # 4 banks
                        for c4 in range(4):
                            for dt_ in range(DT):
                                nc.tensor.matmul(
                                    ps[:, c4 * 512:(c4 + 1) * 512],
                                    Wr[:, dt_ * 1536 + cbl * P:
                                       dt_ * 1536 + (cbl + 1) * P],
                                    xTr[:, dt_ * S + c4 * 512:
                                        dt_ * S + (c4 + 1) * 512],
                                    start=(dt_ == 0), stop=(dt_ == DT - 1))
                        if cb < 16:
                            sbt = qksb.tile([P, S], F32R, tag="qko")
                        else:
                            sbt = qksb.tile([P, S], F32, tag="qko")
                        nc.scalar.activation(sbt[:], ps[:], AF.Identity,
                                             bias=bcols[:, cb:cb + 1])
                        if cb < 8:
                            dst = qT_dr[cb * P:(cb + 1) * P, :]
                        elif cb < 16:
                            dst = kT_dr[(cb - 8) * P:(cb - 7) * P, :]
                        else:
                            dst = vT_dr[(cb - 16) * P:(cb - 15) * P, :]
                        nc.sync.dma_start(dst, sbt[:])

        # ---------------- Phase 2: conv + transposes of C, D2 ----------------
        with tc.tile_pool(name="vst", bufs=2) as vst, \
             tc.tile_pool(name="convp", bufs=2) as convp, \
             tc.tile_pool(name="stg", bufs=4) as stgp, \
             tc.tile_pool(name="tpps", bufs=4, space="PSUM") as tpps:
            for dt_ in range(DT):
                vt = vst.tile([P, S], F32, tag="vt")
                nc.sync.dma_start(vt[:], vT_dr[dt_ * P:(dt_ + 1) * P, :])
                w0 = wcols[:, dt_ * 3 + 0:dt_ * 3 + 1]
                w1 = wcols[:, dt_ * 3 + 1:dt_ * 3 + 2]
                w2 = wcols[:, dt_ * 3 + 2:dt_ * 3 + 3]
                tmp = convp.tile([P, S], F32, tag="tmp")
                d2t = convp.tile([P, S], F32, tag="d2t")
                ct = convp.tile([P, S], F32, tag="ct")
                nc.vector.tensor_scalar_mul(tmp[:], vt[:], w1)
                nc.vector.scalar_tensor_tensor(
                    d2t[:, 1:S], vt[:, 0:S - 1], w0, tmp[:, 1:S],
                    OP.mult, OP.add)
                nc.vector.tensor_copy(d2t[:, 0:1], tmp[:, 0:1])
                nc.vector.scalar_tensor_tensor(
                    ct[:, 0:S - 1], vt[:, 1:S], w2, d2t[:, 0:S - 1],
                    OP.mult, OP.add)
                nc.vector.tensor_copy(ct[:, S - 1:S], d2t[:, S - 1:S])
                for dst_dr, src in ((C_dr, ct), (D2_dr, d2t)):
                    for sg in range(4):
                        ps = tpps.tile([P, 512], F32, tag="tp")
                        for q in range(4):
                            st = sg * 4 + q
                            nc.tensor.transpose(
                                ps[:, q * P:(q + 1) * P],
                                src[:, st * P:(st + 1) * P], ident[:])
                        stg = stgp.tile([P, 512], F32R, tag="stg")
                        nc.scalar.activation(stg[:], ps[:], AF.Identity,
                                             bias=0.0)
                        for q in range(4):
                            st = sg * 4 + q
                            nc.sync.dma_start(
                                dst_dr[st * P:(st + 1) * P,
                                       dt_ * P:(dt_ + 1) * P],
                                stg[:, q * P:(q + 1) * P])

        # ---------------- Phase 3: attention main loop ----------------
        with tc.tile_pool(name="ksb", bufs=1) as ksbp, \
             tc.tile_pool(name="csb", bufs=1) as csbp:
            kTr = ksbp.tile([P, DT * S], F32R)
            for dt_ in range(DT):
                nc.sync.dma_start(kTr[:, dt_ * S:(dt_ + 1) * S],
                                  kT_dr[dt_ * P:(dt_ + 1) * P, :])
            C_sb = csbp.tile([P, ST * D], F32R)
            for st in range(ST):
                nc.sync.dma_start(C_sb[:, st * D:(st + 1) * D],
                                  C_dr[st * P:(st + 1) * P, :])

            with tc.tile_pool(name="qch", bufs=2) as qchp, \
                 tc.tile_pool(name="expp", bufs=18) as expp, \
                 tc.tile_pool(name="frp", bufs=3) as frp, \
                 tc.tile_pool(name="d2p", bufs=2) as d2p, \
                 tc.tile_pool(name="outp", bufs=4) as outp, \
                 tc.tile_pool(name="rdp", bufs=2) as rdp, \
                 tc.tile_pool(name="sps", bufs=2, space="PSUM") as spsp, \
                 tc.tile_pool(name="avps", bufs=2, space="PSUM") as avpsp, \
                 tc.tile_pool(name="denps", bufs=2, space="PSUM") as denpsp:
                for I in range(NI):
                    qch = qchp.tile([P, DT * IC], F32R, tag="qch")
                    for dt_ in range(DT):
                        nc.sync.dma_start(
                            qch[:, dt_ * IC:(dt_ + 1) * IC],
                            qT_dr[dt_ * P:(dt_ + 1) * P, I * IC:(I + 1) * IC])
                    jmax = min(2 * I + 2, ST)
                    exps = []
                    for jt in range(jmax):
                        ps_s = spsp.tile([P, IC], F32, tag="sps")
                        for dt_ in range(DT):
                            nc.tensor.matmul(
                                ps_s[:],
                                kTr[:, dt_ * S + jt * P:dt_ * S + (jt + 1) * P],
                                qch[:, dt_ * IC:(dt_ + 1) * IC],
                                start=(dt_ == 0), stop=(dt_ == DT - 1))
                        et = expp.tile([P, IC], F32R, tag="et")
                        nc.scalar.activation(et[:], ps_s[:], AF.Exp,
                                             scale=SCALE)
                        exps.append(et)
                    # frontier diagonal tiles: strict (j<i) and diag-only parts
                    fr = {}
                    for ft in (2 * I, 2 * I + 1):
                        sub = ft - 2 * I
                        stt = frp.tile([P, P], F32R, tag="fstrict")
                        dgt = frp.tile([P, P], F32R, tag="fdiag")
                        nc.vector.tensor_mul(
                            stt[:], exps[ft][:, sub * P:(sub + 1) * P],
                            strictm[:])
                        nc.vector.tensor_mul(
                            dgt[:], exps[ft][:, sub * P:(sub + 1) * P],
                            identm[:])
                        fr[ft] = (stt, dgt)
                    for sub in range(2):
                        it = 2 * I + sub
                        d2s = d2p.tile([P, D], F32R, tag="d2s")
                        nc.sync.dma_start(d2s[:], D2_dr[it * P:(it + 1) * P, :])
                        ps_o = avpsp.tile([P, D], F32, tag="avps")  # 2 banks
                        ps_d = denpsp.tile([P, 2], F32, tag="denps")
                        stt, dgt = fr[it]
                        # (lhsT, rhs_base) sequence: full tiles, strict, diag
                        seq = [(exps[jt][:, sub * P:(sub + 1) * P],
                                C_sb, jt) for jt in range(it)]
                        seq.append((stt[:], C_sb, it))
                        nseq = len(seq)
                        for idx, (lhsT, rhs_sb, jt) in enumerate(seq):
                            first = idx == 0
                            for hh in range(2):
                                nc.tensor.matmul(
                                    ps_o[:, hh * 512:(hh + 1) * 512],
                                    lhsT,
                                    rhs_sb[:, jt * D + hh * 512:
                                           jt * D + (hh + 1) * 512],
                                    start=first, stop=False)
                            nc.tensor.matmul(ps_d[:], lhsT, ones_r[:],
                                             start=first, stop=False)
                        for hh in range(2):
                            nc.tensor.matmul(
                                ps_o[:, hh * 512:(hh + 1) * 512],
                                dgt[:],
                                d2s[:, hh * 512:(hh + 1) * 512],
                                start=False, stop=True)
                        nc.tensor.matmul(ps_d[:], dgt[:], ones_r[:],
                                         start=False, stop=True)
                        rdt = rdp.tile([P, 1], F32, tag="rdt")
                        nc.vector.reciprocal(rdt[:], ps_d[:, 0:1])
                        for hh in range(2):
                            ost = outp.tile([P, 512], F32, tag="ost")
                            nc.vector.tensor_scalar_mul(
                                ost[:], ps_o[:, hh * 512:(hh + 1) * 512],
                                rdt[:])
                            nc.sync.dma_start(
                                out_d[it * P:(it + 1) * P,
                                      hh * 512:(hh + 1) * 512], ost[:])
    nc.compile()
    return nc


_NC_CACHE = None


def _get_nc():
    global _NC_CACHE
    if _NC_CACHE is None:
        _NC_CACHE = build()
    return _NC_CACHE


def kernel(x, W_qkv, b_qkv, W_vconv):
    B = x.shape[0]
    nc = _get_nc()
    in_maps = []
    for bb in range(B):
        in_maps.append({
            "x": np.ascontiguousarray(x[bb], dtype=np.float32),
            "W": np.ascontiguousarray(W_qkv, dtype=np.float32),
            "b": np.ascontiguousarray(b_qkv, dtype=np.float32),
            "wc": np.ascontiguousarray(W_vconv, dtype=np.float32),
        })
    r = run_bass_kernel_spmd(nc, in_maps, core_ids=list(range(B)))
    return np.stack([r.results[bb]["out"] for bb in range(B)], axis=0)
